# revision 32
# baseline (speedup 1.0000x reference)
"""Trainium2 Bass kernel for nn_AngularMultiCenterEmotionBall.

Data-parallel over batch B=16384 across 8 NeuronCores (2048 rows/core).
Each core computes, for its batch shard:
  - LayerNorm(z): stats via bn_stats on row-major z; the (z - mu) @ W
    product is computed from a host-pretransposed zT feed with the mean
    folded in as a rank-1 correction row of the matmul, and the 1/std
    scale folded into the PSUM->SBUF copies. gamma/beta are folded into
    the projection weights on the host.
  - one fused bf16 matmul z0 @ [W_sh | W_sp | W_sh @ c_norm.T] (1024x1052)
  - row norms of z_sh, per-sample center sims -> softmax q, relu(dist_w-r_w)
  - segment statistics (sum_q, sum q*log q, counts) via one-hot mask matmuls
  - partial cross-correlation  z_sh.T @ z_sp  [768, 256]
  - partial column sums/squares of z_sh, z_sp
The host sums the 8 partial outputs and finishes the tiny scalar math
(plus the centers-only overlap/diversity losses).
"""

import os
import sys

import numpy as np

sys.path.insert(0, "/opt/trn_rl_repo")

# problem constants (hardcoded per harness contract)
B, ZD, C, K = 16384, 1024, 7, 4
DSH, DSP = 768, 256
TAU = 0.15
NCORES = 8
BL = B // NCORES          # 2048 rows per core
P = 128
NT = BL // P              # 16 row-tiles per core
CK = C * K                # 28
NW = DSH + DSP + CK       # 1052 fused output columns
KC = ZD // P              # 8 contraction chunks

_GRAPH_CACHE = {}


def _split_multiwaits(nc):
    """Walrus codegen in this container accepts at most one semaphore wait
    per engine instruction. TileContext attaches several. Peel the extra
    waits off into standalone single-wait EventSemaphore instructions
    (what raw-bass wait_ge emits) placed just before the instruction —
    the engine is in-order, so wait(A); wait(B); op == op waiting {A,B}.
    Applied as a JSON rewrite at serialization time."""
    import json

    orig = nc.to_json_bytes

    def patched():
        d = json.loads(orig())
        ctr = [0]
        for f in d["functions"]:
            for b in f["blocks"]:
                insts = b.get("instructions")
                if not insts:
                    continue
                out = []
                for i in insts:
                    si = i.get("sync_info") or {}
                    waits = si.get("on_wait") or []
                    if len(waits) > 1:
                        for w in waits[:-1]:
                            ctr[0] += 1
                            out.append(
                                {
                                    "engine": i["engine"],
                                    "ins": [],
                                    "name": f"splitwait_{ctr[0]}",
                                    "opcode": "EventSemaphore",
                                    "outs": [],
                                    "sync_info": {
                                        "on_update": [],
                                        "on_wait": [w],
                                    },
                                }
                            )
                        si["on_wait"] = [waits[-1]]
                    out.append(i)
                b["instructions"] = out
        return json.dumps(d).encode()

    nc.to_json_bytes = patched
    return nc


def _build_graph(with_bias: bool):
    import concourse.bass as bass
    import concourse.tile as tile
    from concourse import mybir
    from concourse.masks import make_identity

    f32 = mybir.dt.float32
    b16 = mybir.dt.bfloat16
    AF = mybir.ActivationFunctionType
    ALU = mybir.AluOpType

    nc = bass.Bass()
    z_ext = nc.declare_dram_parameter("z", [BL, ZD], b16, isOutput=False)
    zt_ext = nc.declare_dram_parameter("zt", [NT, ZD, P], b16, isOutput=False)
    w_ext = nc.declare_dram_parameter("w", [9 * P, NW], b16, isOutput=False)
    mk_ext = nc.declare_dram_parameter("mk", [BL, 8], f32, isOutput=False)
    rl_ext = nc.declare_dram_parameter("rl", [BL, K], f32, isOutput=False)
    if with_bias:
        br_ext = nc.declare_dram_parameter("br", [1, NW], f32, isOutput=False)
    o_corr = nc.declare_dram_parameter("o_corr", [DSP, DSH], f32, isOutput=True)
    o_a0 = nc.declare_dram_parameter("o_a0", [1, 512], f32, isOutput=True)
    o_a1 = nc.declare_dram_parameter("o_a1", [1, 512], f32, isOutput=True)
    o_misc = nc.declare_dram_parameter("o_misc", [8, 272], f32, isOutput=True)
    o_intra = nc.declare_dram_parameter("o_intra", [P, NT], f32, isOutput=True)

    with tile.TileContext(nc) as tc:
        with (
            tc.tile_pool(name="singles", bufs=1) as singles,
            tc.tile_pool(name="work", bufs=2) as work,
            tc.tile_pool(name="zin", bufs=4) as zin,
            tc.tile_pool(name="stats", bufs=6) as stats,
            tc.tile_pool(name="outst", bufs=2) as outst,
            tc.tile_pool(name="pmain", bufs=1, space="PSUM") as pmain,
            tc.tile_pool(name="ptr", bufs=2, space="PSUM") as ptr_pool,
            tc.tile_pool(name="pacc", bufs=1, space="PSUM") as pacc,
        ):
            # ---- persistent SBUF state ----
            W_sb = singles.tile([P, 9, NW], b16)
            for kc in range(9):
                nc.scalar.dma_start(
                    out=W_sb[:, kc, :],
                    in_=w_ext[kc * P : (kc + 1) * P, :],
                )
            zT_all = singles.tile([P, KC, BL], b16)
            mask_all = singles.tile([P, NT, 8], f32)
            nc.gpsimd.dma_start(
                out=mask_all, in_=mk_ext[:].rearrange("(t p) c -> p t c", p=P)
            )
            rlab_all = singles.tile([P, NT, K], f32)
            nc.gpsimd.dma_start(
                out=rlab_all, in_=rl_ext[:].rearrange("(t p) k -> p t k", p=P)
            )
            if with_bias:
                br_sb = singles.tile([1, NW], f32)
                nc.sync.dma_start(out=br_sb, in_=br_ext[:])

            ident = singles.tile([P, P], f32)
            make_identity(nc, ident)
            ident_bf = singles.tile([P, P], b16)
            nc.scalar.copy(out=ident_bf, in_=ident)
            eps_t = singles.tile([P, 1], f32)
            nc.gpsimd.memset(eps_t, 1e-5)
            zero_t = singles.tile([P, 1], f32)
            nc.gpsimd.memset(zero_t, 0.0)
            eps8_t = singles.tile([P, 1], f32)
            nc.gpsimd.memset(eps8_t, 1e-8)
            one_t = singles.tile([P, 1], f32)
            nc.gpsimd.memset(one_t, 1.0)
            ones_col = singles.tile([P, 1], b16)
            nc.scalar.copy(out=ones_col, in_=one_t)
            mask_bf = singles.tile([P, NT, 8], b16)
            nc.scalar.copy(out=mask_bf, in_=mask_all)

            z_sh_all = singles.tile([P, NT, DSH], b16)
            z_sp_all = singles.tile([P, NT, DSP], b16)
            sraw_all = singles.tile([P, NT, CK], f32)
            n2_all = singles.tile([P, NT], f32)
            negmu_sb = singles.tile([1, BL], b16)

            # persistent PSUM accumulators. NOTE: regions that accumulate
            # concurrently (interleaved start..stop groups) must be in
            # distinct banks — a start=True clears has_written for the
            # whole bank, so a second group sharing the bank would turn
            # the other group's next accumulate into an overwrite.
            acc0 = pacc.tile([1, 512], f32)    # ssq_sh[0:512]
            acc1 = pacc.tile([1, 512], f32)    # ssq_sh[512:768]
            acc2 = pacc.tile([8, 512], f32)    # [0,0:256]=sum_sp ; [0:8,256:262]=seg (tail)

            def copy_scaled(dst, src_psum, rstd, col0, ncol, engine):
                """PSUM->SBUF move with the folded 1/std LayerNorm scale
                (plus the folded bias row when present)."""
                if engine == "act":
                    nc.scalar.activation(
                        out=dst, in_=src_psum, func=AF.Copy, scale=rstd
                    )
                else:
                    nc.vector.tensor_scalar_mul(dst, src_psum, rstd)
                if with_bias:
                    nc.vector.tensor_tensor(
                        out=dst,
                        in0=dst,
                        in1=br_sb[0:1, col0 : col0 + ncol].partition_broadcast(P),
                        op=ALU.add,
                    )

            # ---- main loop, software-pipelined by one tile ----
            # stats(t+1) and ssq-accumulation(t-1) are emitted around
            # tile t's matmul group so the in-order PE stream never
            # waits on the bn-stats chain or the ACT squares.
            rstds = [None] * NT
            sqhs = [None] * NT

            def emit_ztload(t):
                nc.sync.dma_start(
                    out=zT_all[:, :, t * P : (t + 1) * P],
                    in_=zt_ext[t].rearrange("(o p) b -> p o b", p=P),
                )

            def emit_stats(t):
                ts_ = slice(t * P, (t + 1) * P)
                zt = zin.tile([P, ZD], b16, name="zt")
                nc.gpsimd.dma_start(out=zt, in_=z_ext[ts_, :])
                st = stats.tile([P, 2, 6], f32, name="st")
                nc.vector.bn_stats(out=st[:, 0, :], in_=zt[:, 0:512])
                nc.vector.bn_stats(out=st[:, 1, :], in_=zt[:, 512:1024])
                mv = stats.tile([P, 2], f32, name="mv")
                nc.vector.bn_aggr(out=mv, in_=st)
                stdt = stats.tile([P, 1], f32, name="stdt")
                nc.scalar.activation(
                    out=stdt, in_=mv[:, 1:2], func=AF.Sqrt, bias=eps_t, scale=1.0
                )
                rstd = stats.tile([P, 1], f32, name="rstd")
                nc.vector.reciprocal(out=rstd, in_=stdt)
                rstds[t] = rstd
                mub = stats.tile([P, 1], b16, name="mub")
                nc.scalar.activation(
                    out=mub, in_=mv[:, 0:1], func=AF.Copy, scale=-1.0
                )
                ptr_mu = ptr_pool.tile([P, 512], b16, tag="tr", name="ptr_mu")
                nc.tensor.transpose(ptr_mu[0:1, 0:P], mub, ident_bf)
                nc.scalar.copy(out=negmu_sb[0:1, ts_], in_=ptr_mu[0:1, 0:P])

            def emit_mm(t):
                ts_ = slice(t * P, (t + 1) * P)
                pA = pmain.tile([P, 512], f32, tag="mA", name="pA")
                pB = pmain.tile([P, 512], f32, tag="mB", name="pB")
                pC = pmain.tile([P, CK], f32, tag="mC", name="pC")
                for kc in range(KC):
                    lhsT = zT_all[:, kc, ts_]
                    first = kc == 0
                    nc.tensor.matmul(
                        pA, lhsT, W_sb[:, kc, 0:512], start=first, stop=False
                    )
                    nc.tensor.matmul(
                        pB, lhsT, W_sb[:, kc, 512:1024], start=first, stop=False
                    )
                    nc.tensor.matmul(
                        pC, lhsT, W_sb[:, kc, 1024:NW], start=first, stop=False
                    )
                # rank-1 LayerNorm mean correction: += (-mu) x colsum(W)
                cmu = negmu_sb[0:1, ts_]
                nc.tensor.matmul(
                    pA, cmu, W_sb[0:1, 8, 0:512], start=False, stop=True
                )
                nc.tensor.matmul(
                    pB, cmu, W_sb[0:1, 8, 512:1024], start=False, stop=True
                )
                nc.tensor.matmul(
                    pC, cmu, W_sb[0:1, 8, 1024:NW], start=False, stop=True
                )
                return pA, pB, pC

            def emit_copies(t, pA, pB, pC):
                rstd = rstds[t]
                copy_scaled(z_sh_all[:, t, 0:512], pA, rstd, 0, 512, "act")
                copy_scaled(z_sh_all[:, t, 512:768], pB[:, 0:256], rstd, 512, 256, "dve")
                copy_scaled(z_sp_all[:, t, :], pB[:, 256:512], rstd, 768, 256, "dve")
                copy_scaled(sraw_all[:, t, :], pC, rstd, 1024, CK, "act")
                sqh = work.tile([P, DSH], b16, tag="sqh", name="sqh")
                nc.scalar.activation(
                    out=sqh, in_=z_sh_all[:, t, :], func=AF.Square,
                    bias=zero_t,
                    accum_out=n2_all[:, t : t + 1],
                )
                sqhs[t] = sqh

            def emit_ssq(t):
                fl = t == 0
                ll = t == NT - 1
                sqh = sqhs[t]
                nc.tensor.matmul(
                    acc0[0:1, :], ones_col, sqh[:, 0:512],
                    start=fl, stop=ll, skip_group_check=True,
                )
                nc.tensor.matmul(
                    acc1[0:1, 0:256], ones_col, sqh[:, 512:768],
                    start=fl, stop=ll, skip_group_check=True,
                )
                nc.tensor.matmul(
                    acc2[0:1, 0:256], ones_col, z_sp_all[:, t, :],
                    start=fl, stop=ll, skip_group_check=True,
                )

            for t0 in range(4):
                emit_ztload(t0)
            emit_stats(0)
            emit_stats(1)
            for t in range(NT):
                if t >= 1:
                    emit_ssq(t - 1)
                mm = emit_mm(t)
                if t + 4 < NT:
                    emit_ztload(t + 4)
                if t + 2 < NT:
                    emit_stats(t + 2)
                emit_copies(t, *mm)
            emit_ssq(NT - 1)

            # ---- batched softmax / loss tail over [128, 16, *] ----
            nrm = stats.tile([P, NT], f32, tag="nrm")
            nc.scalar.activation(out=nrm, in_=n2_all, func=AF.Sqrt, bias=zero_t)
            nc.vector.tensor_scalar_max(nrm, nrm, 1e-12)
            rn = stats.tile([P, NT], f32, tag="rn")
            nc.vector.reciprocal(out=rn, in_=nrm)

            sim_all = singles.tile([P, NT, CK], f32)
            nc.vector.tensor_tensor(
                out=sim_all, in0=sraw_all,
                in1=rn[:, :, None].to_broadcast([P, NT, CK]), op=ALU.mult,
            )
            # gather label class: simK[p,t,k] = sum_c mask[p,t,c] * sim[p,t,c*4+k]
            t47 = singles.tile([P, NT, K, C], f32)
            nc.vector.tensor_tensor(
                out=t47,
                in0=sim_all.rearrange("p t (c k) -> p t k c", k=K),
                in1=mask_all[:, :, None, 0:C].to_broadcast([P, NT, K, C]),
                op=ALU.mult,
            )
            simK = singles.tile([P, NT, K], f32)
            nc.vector.reduce_sum(out=simK, in_=t47, axis=mybir.AxisListType.X)

            mx = stats.tile([P, NT], f32, tag="mx")
            nc.vector.reduce_max(out=mx, in_=simK, axis=mybir.AxisListType.X)
            dsub = singles.tile([P, NT, K], f32)
            nc.vector.tensor_tensor(
                out=dsub, in0=simK,
                in1=mx[:, :, None].to_broadcast([P, NT, K]), op=ALU.subtract,
            )
            e_all = singles.tile([P, NT, K], f32)
            nc.scalar.activation(
                out=e_all, in_=dsub, func=AF.Exp, scale=1.0 / TAU, bias=zero_t
            )
            se = stats.tile([P, NT], f32, tag="se")
            nc.vector.reduce_sum(out=se, in_=e_all, axis=mybir.AxisListType.X)
            rse = stats.tile([P, NT], f32, tag="rse")
            nc.vector.reciprocal(out=rse, in_=se)
            q_all = singles.tile([P, NT, K], f32)
            nc.vector.tensor_tensor(
                out=q_all, in0=e_all,
                in1=rse[:, :, None].to_broadcast([P, NT, K]), op=ALU.mult,
            )
            lg = singles.tile([P, NT, K], f32)
            nc.scalar.activation(out=lg, in_=q_all, func=AF.Ln, bias=eps8_t)
            ql = singles.tile([P, NT, K], f32)
            nc.vector.tensor_tensor(out=ql, in0=q_all, in1=lg, op=ALU.mult)
            qlsum = stats.tile([P, NT], f32, tag="qlsum")
            nc.vector.reduce_sum(out=qlsum, in_=ql, axis=mybir.AxisListType.X)

            qs = singles.tile([P, NT, K], f32, tag="qs")
            nc.vector.tensor_tensor(out=qs, in0=q_all, in1=simK, op=ALU.mult)
            ds = stats.tile([P, NT], f32, tag="ds")
            nc.vector.reduce_sum(out=ds, in_=qs, axis=mybir.AxisListType.X)
            qr = singles.tile([P, NT, K], f32, tag="qr")
            nc.vector.tensor_tensor(out=qr, in0=q_all, in1=rlab_all, op=ALU.mult)
            rw = stats.tile([P, NT], f32, tag="rw")
            nc.vector.reduce_sum(out=rw, in_=qr, axis=mybir.AxisListType.X)
            s_all = stats.tile([P, NT], f32, tag="s_all")
            nc.vector.tensor_tensor(out=s_all, in0=ds, in1=rw, op=ALU.add)
            strip = singles.tile([P, NT], f32)
            # relu(dist_w - r_w) = Relu(1 - (ds + rw))
            nc.scalar.activation(
                out=strip, in_=s_all, func=AF.Relu, scale=-1.0, bias=one_t
            )
            nc.sync.dma_start(out=o_intra[:], in_=strip)

            # ---- tail: ssq_sp via sequential accumulation (own tr bank) ----
            psp_full = ptr_pool.tile([P, 512], f32, tag="tr")
            for t in range(NT):
                sqp = work.tile([P, DSP], b16, tag="sqp")
                nc.scalar.activation(
                    out=sqp, in_=z_sp_all[:, t, :], func=AF.Square, bias=zero_t
                )
                nc.tensor.matmul(
                    psp_full[0:1, 0:256], ones_col, sqp,
                    start=(t == 0), stop=(t == NT - 1), skip_group_check=True,
                )
            a1s = outst.tile([1, 512], f32, tag="a1s")
            nc.scalar.copy(out=a1s[0:1, 0:256], in_=acc1[0:1, 0:256])
            nc.scalar.copy(out=a1s[0:1, 256:512], in_=psp_full[0:1, 0:256])
            nc.sync.dma_start(out=o_a1[:], in_=a1s)

            # ---- cross-correlation tail (transposed layout):
            # corrT[j, i] = sum_b z_sp[b, j] * z_sh[b, i]; z_sp chunks are
            # the stationary operand so the moving stream is 512 wide.
            for jc in range(DSP // P):
                pj0 = ptr_pool.tile([P, 512], f32, tag="tr", name="pj0")
                pj1_full = ptr_pool.tile([P, 512], f32, tag="tr", name="pj1")
                pj1 = pj1_full[:, 0:256]
                for t in range(NT):
                    lhsT = z_sp_all[:, t, jc * P : (jc + 1) * P]
                    nc.tensor.matmul(
                        pj0, lhsT, z_sh_all[:, t, 0:512],
                        start=(t == 0), stop=(t == NT - 1),
                        skip_group_check=True,
                    )
                    nc.tensor.matmul(
                        pj1, lhsT, z_sh_all[:, t, 512:768],
                        start=(t == 0), stop=(t == NT - 1),
                        skip_group_check=True,
                    )
                ct = outst.tile([P, DSH], f32, tag="ct")
                nc.scalar.copy(out=ct[:, 0:512], in_=pj0)
                nc.scalar.copy(out=ct[:, 512:768], in_=pj1)
                nc.sync.dma_start(out=o_corr[jc * P : (jc + 1) * P, :], in_=ct)

            # segment-sum matmuls: acc2[c, 256+j] += sum_b mask[b,c]*R[b,j]
            R_all = singles.tile([P, NT, 6], b16)
            nc.scalar.copy(out=R_all[:, :, 0:4], in_=q_all)
            nc.scalar.copy(out=R_all[:, :, 4:5], in_=qlsum[:, :, None])
            nc.scalar.copy(
                out=R_all[:, :, 5:6],
                in_=one_t[:, None, 0:1].to_broadcast([P, NT, 1]),
            )
            for t in range(NT):
                nc.tensor.matmul(
                    acc2[:, 256:262],
                    mask_bf[:, t, :],
                    R_all[:, t, :],
                    start=(t == 0), stop=(t == NT - 1),
                    skip_group_check=True,
                )

            # ---- epilogue: accumulators -> SBUF -> DRAM ----
            a0s = outst.tile([1, 512], f32, tag="a0s")
            nc.scalar.copy(out=a0s, in_=acc0)
            nc.sync.dma_start(out=o_a0[:], in_=a0s)
            ms = outst.tile([8, 272], f32, tag="ms")
            nc.gpsimd.memset(ms, 0.0)
            nc.scalar.copy(out=ms[0:1, 0:256], in_=acc2[0:1, 0:256])
            nc.scalar.copy(out=ms[:, 256:262], in_=acc2[:, 256:262])
            nc.sync.dma_start(out=o_misc[:], in_=ms)

    return _split_multiwaits(nc)


def _host_prep(inputs):
    import ml_dtypes

    bf16 = ml_dtypes.bfloat16
    z = np.asarray(inputs["z"], dtype=np.float32)
    labels = np.asarray(inputs["labels"]).astype(np.int64)
    gamma = np.asarray(inputs["ln_gamma"], dtype=np.float32)
    beta = np.asarray(inputs["ln_beta"], dtype=np.float32)
    W_sh = np.asarray(inputs["W_sh"], dtype=np.float32)
    b_sh = np.asarray(inputs["b_sh"], dtype=np.float32)
    W_sp = np.asarray(inputs["W_sp"], dtype=np.float32)
    b_sp = np.asarray(inputs["b_sp"], dtype=np.float32)
    centers = np.asarray(inputs["centers"], dtype=np.float32)
    radii = np.asarray(inputs["ema_radii"], dtype=np.float32)

    cf = centers.reshape(CK, DSH)
    cn = cf / np.maximum(
        np.linalg.norm(cf, axis=1, keepdims=True), 1e-12
    ).astype(np.float32)
    W_all = np.concatenate([W_sh, W_sp, W_sh @ cn.T], axis=1)  # [ZD, NW]
    W_eff = (gamma[:, None] * W_all).astype(np.float32)
    # row 1024: column sums for the rank-1 (-mu) LayerNorm correction;
    # rows 1025..1151: zero pad to 9*128 for the [128, 9, NW] SBUF layout.
    W_ext = np.zeros((9 * P, NW), np.float32)
    W_ext[:ZD] = W_eff
    W_ext[ZD] = W_eff.sum(0)
    W_bf = np.ascontiguousarray(W_ext.astype(bf16))

    be_sh = beta @ W_sh + b_sh
    be_sp = beta @ W_sp + b_sp
    b_eff = np.concatenate([be_sh, be_sp, be_sh @ cn.T]).astype(np.float32)
    with_bias = bool(np.any(b_eff != 0.0))

    onehot = (labels[:, None] == np.arange(8)[None, :]).astype(np.float32)
    rlab = radii.reshape(CK // K, K)[labels].astype(np.float32)  # [B, K]
    z_bf = z.astype(bf16)

    in_maps = []
    for i in range(NCORES):
        sl = slice(i * BL, (i + 1) * BL)
        m = {
            "z": np.ascontiguousarray(z_bf[sl]),
            "zt": np.ascontiguousarray(
                z_bf[sl].T.reshape(ZD, NT, P).transpose(1, 0, 2)
            ),
            "w": W_bf,
            "mk": np.ascontiguousarray(onehot[sl]),
            "rl": np.ascontiguousarray(rlab[sl]),
        }
        if with_bias:
            m["br"] = np.ascontiguousarray(b_eff[None, :])
        in_maps.append(m)
    return in_maps, with_bias, cn


def _host_finish(results, cn):
    f64 = np.float64
    corr_raw = np.zeros((DSH, DSP), f64)
    a0 = np.zeros(512, f64)
    a1 = np.zeros(512, f64)
    sum_sp = np.zeros(DSP, f64)
    seg = np.zeros((8, 6), f64)
    intra_sum = 0.0
    for r in results:
        corr_raw += r["o_corr"].T.astype(f64)
        a0 += r["o_a0"][0].astype(f64)
        a1 += r["o_a1"][0].astype(f64)
        sum_sp += r["o_misc"][0, 0:256].astype(f64)
        seg += r["o_misc"][:, 256:262].astype(f64)
        intra_sum += float(r["o_intra"].astype(f64).sum())

    ssq_sh = np.concatenate([a0, a1[0:256]])
    ssq_sp = a1[256:512]
    sum_q = seg[0:C, 0:K]
    qlsum_c = seg[0:C, 4]
    counts = seg[0:C, 5]

    n_sh = np.maximum(np.sqrt(ssq_sh), 1e-12)
    n_sp = np.maximum(np.sqrt(ssq_sp), 1e-12)
    corr = corr_raw / np.outer(n_sh, n_sp)
    L_ortho = (corr**2).mean()

    v = ssq_sp / B - (sum_sp / B) ** 2
    L_var = np.maximum(0.05 - v, 0.0).mean()

    L_intra = intra_sum / B

    p = sum_q / (sum_q.sum(-1, keepdims=True) + 1e-8)
    H_marg = -(p * np.log(p + 1e-8)).sum(-1)
    H_cond = (-qlsum_c) / np.maximum(counts, 1.0)
    valid = counts > 0
    L_bal_k = np.log(f64(K)) - H_marg + H_cond
    L_balance = np.where(valid, L_bal_k, 0.0).sum() / max(int(valid.sum()), 1)

    sim_mat = (cn @ cn.T).astype(f64)
    blkmask = 1.0 - np.kron(np.eye(C), np.ones((K, K)))
    L_overlap = (np.maximum(sim_mat - 0.3, 0.0) * blkmask).sum() / (
        blkmask.sum() + 1e-6
    )
    cnr = cn.reshape(C, K, DSH).astype(f64)
    sims_in = np.einsum("ckd,cld->ckl", cnr, cnr)
    triu = np.triu(np.ones((K, K)), 1)
    L_div = (np.maximum(sims_in - 0.8, 0.0) * triu).sum() / max(
        C * K * (K - 1) // 2, 1
    )

    L_ball = L_intra + 0.3 * L_overlap + 0.2 * L_div + 0.15 * L_balance
    loss = L_ball + 0.02 * L_ortho + 0.005 * L_var
    return np.float32(loss)


def _run_hw(nc, in_maps, trace=False, tmpdir=None):
    from concourse.bass_utils import run_bass_kernel_spmd

    res = run_bass_kernel_spmd(
        nc, in_maps, core_ids=list(range(NCORES)), trace=trace, tmpdir=tmpdir
    )
    return res


def _run_sim(nc, in_maps):
    from concourse.bass_interp import CoreSim

    outs = []
    for i, im in enumerate(in_maps):
        sim = CoreSim(nc, publish_trace=False)
        sim.assign_tensors(im)
        sim.simulate()
        outs.append(
            {k: np.array(sim.tensor(k)) for k in
             ("o_corr", "o_a0", "o_a1", "o_misc", "o_intra")}
        )
    return outs


def kernel(**inputs) -> np.ndarray:
    in_maps, with_bias, cn = _host_prep(inputs)
    if with_bias not in _GRAPH_CACHE:
        _GRAPH_CACHE[with_bias] = _build_graph(with_bias)
    nc = _GRAPH_CACHE[with_bias]
    if os.environ.get("KERNEL_BASS_SIM"):
        results = _run_sim(nc, in_maps)
    else:
        results = _run_hw(nc, in_maps).results
    return _host_finish(results, cn)
